# revision 13
# baseline (speedup 1.0000x reference)
"""Trainium2 Bass kernel for nn_AutoregressiveDecoder.

Strategy
--------
* Math: the reference's full re-decode per step is exactly equivalent to
  incremental (KV-cached) decoding; additionally the encoder K/V projections
  fold into the query / output side of cross-attention:
      logits = x @ [(Wq/sqrt(KD)) @ Wk^T] @ Enc^T       (bk is softmax-shift invariant)
      ctx @ Wo = (A @ Enc) @ [Wv @ Wo]                  (bv folds via sum(A)=1, then LN-shift)
  so encoder K/V are never materialized; only Enc itself stays SBUF-resident
  (~3 MB/core in bf16).  A ones-column appended to Enc makes the matmul that
  computes A@Enc also produce the softmax denominators.
* Sharding: pure data parallel, batch 16 -> 2 per core x 8 cores, weights
  replicated, no collectives.
* Layouts: natural residual stream x [2, E] (bn_stats LayerNorm); transposed
  activations for matmul contraction produced by a tiny "x-stationary vs 2x2
  identity" matmul; q/k/v/MLP-hidden computed directly transposed
  (feature-on-partition) with bf16 weight tiles stationary (fast weight load).
"""
import os
import sys

import numpy as np

for _p in ("/opt/trn_rl_repo", "/root/.axon_site/_ro/trn_rl_repo"):
    if os.path.isdir(_p) and _p not in sys.path:
        sys.path.insert(0, _p)

from contextlib import ExitStack  # noqa: E402

import ml_dtypes  # noqa: E402

import concourse.bass as bass  # noqa: E402
import concourse.tile as tile  # noqa: E402
from concourse import bacc, mybir  # noqa: E402
from concourse.bass_utils import run_bass_kernel_spmd  # noqa: E402

B, S, E, H, KD, MLPD, T_MAX, L = 16, 1024, 256, 4, 256, 1024, 10, 2
NCORES = 8
BL = B // NCORES  # 2 batches per core
EPS = 1e-3
ET = E // 128     # contraction tiles over E
F32 = mybir.dt.float32
BF16 = mybir.dt.bfloat16
AF = mybir.ActivationFunctionType
ALU = mybir.AluOpType
BF = ml_dtypes.bfloat16


# --------------------------------------------------------------------------
# host-side weight folding
# --------------------------------------------------------------------------
def _f32(a):
    return np.ascontiguousarray(np.asarray(a, dtype=np.float32))


def _pack_rows(w, nkt):
    """[K, N] -> [128, nkt, N] so tile kt holds rows kt*128..kt*128+127."""
    return np.ascontiguousarray(w.reshape(nkt, 128, -1).transpose(1, 0, 2))


def _prep_weights(params):
    s = np.float32(1.0 / np.sqrt(KD))
    pos = _f32(params["pos"])
    coord_W = _f32(params["coord_W"])
    coord_b = _f32(params["coord_b"])
    out_W = _f32(params["out_W"])
    out_b = _f32(params["out_b"])

    acc = {k: [] for k in ("wq_sa", "wk_sa", "wv_sa", "w_qk", "w1",
                           "wo_sa", "w_vo", "w2", "bq_saT", "b_qkT")}
    gb = {k: [] for k in ("g1", "b1", "g2", "b2", "bf1", "bf2")}
    for lp in params["layers"]:
        sa = {k: _f32(v) for k, v in lp["sa"].items()}
        ca = {k: _f32(v) for k, v in lp["ca"].items()}
        acc["wq_sa"].append(_pack_rows((sa["Wq"] * s).reshape(E, H * KD), ET))
        acc["wk_sa"].append(_pack_rows(sa["Wk"].reshape(E, H * KD), ET))
        acc["wv_sa"].append(_pack_rows(sa["Wv"].reshape(E, H * KD), ET))
        acc["wo_sa"].append(_pack_rows(sa["Wo"].reshape(H * KD, E), H * KD // 128))
        acc["w_qk"].append(_pack_rows(
            np.einsum("ehd,fhd->ehf", ca["Wq"] * s, ca["Wk"]).reshape(E, H * E), ET))
        acc["w_vo"].append(_pack_rows(
            np.einsum("fhd,hde->hfe", ca["Wv"], ca["Wo"]).reshape(H * E, E),
            H * E // 128))
        acc["w1"].append(_pack_rows(_f32(lp["W1"]), ET))
        acc["w2"].append(_pack_rows(_f32(lp["W2"]), MLPD // 128))
        bq = (sa["bq"] * s).reshape(H * KD)
        acc["bq_saT"].append(np.ascontiguousarray(bq.reshape(H * KD // 128, 128).T))
        bqk = np.einsum("hd,fhd->hf", ca["bq"] * s, ca["Wk"]).reshape(H * E)
        acc["b_qkT"].append(np.ascontiguousarray(bqk.reshape(H * E // 128, 128).T))
        for k in gb:
            gb[k].append(_f32(lp[k]))

    out = {k: np.stack(v, 1).astype(BF) for k, v in acc.items()
           if k not in ("bq_saT", "b_qkT")}
    out["bq_saT"] = np.stack(acc["bq_saT"], 1).astype(np.float32)  # [128, L, 8]
    out["b_qkT"] = np.stack(acc["b_qkT"], 1).astype(np.float32)
    out["posc"] = np.ascontiguousarray(np.broadcast_to(
        (pos[:T_MAX] + coord_b[None, :])[None], (BL, T_MAX, E))).astype(np.float32)
    out["outw_t"] = np.ascontiguousarray(
        out_W.reshape(ET, 128, 2).transpose(1, 0, 2)).astype(np.float32)
    out["outb"] = out_b.reshape(2, 1).astype(np.float32)
    out["coordw"] = coord_W.astype(np.float32)
    out["id2"] = np.eye(2, dtype=np.float32)
    out["id4"] = np.eye(4, dtype=BF)

    flags = {
        "g1": [not np.allclose(g, 1.0) for g in gb["g1"]],
        "b1": [bool(np.any(b)) for b in gb["b1"]],
        "g2": [not np.allclose(g, 1.0) for g in gb["g2"]],
        "b2": [bool(np.any(b)) for b in gb["b2"]],
        "bf1": [bool(np.any(b)) for b in gb["bf1"]],
        "bf2": [bool(np.any(b)) for b in gb["bf2"]],
    }
    if any(any(v) for v in flags.values()):
        out["lnaff"] = np.ascontiguousarray(np.broadcast_to(np.stack(
            [np.stack([gb["g1"][l], gb["b1"][l], gb["g2"][l], gb["b2"][l],
                       gb["bf2"][l]]) for l in range(L)])[None],
            (BL, L, 5, E))).astype(np.float32)
        out["bf1T"] = np.stack(
            [np.ascontiguousarray(gb["bf1"][l].reshape(8, 128).T)
             for l in range(L)], 1).astype(np.float32)
    return out, flags


def _prep_enc(enc_core):
    """[BL, S, E] f32 -> enc_t [128, ET, BL, S] bf16 and enc_aug [128, BL, 8, 257] bf16."""
    enc_t = np.ascontiguousarray(
        enc_core.transpose(2, 0, 1).reshape(ET, 128, BL, S).transpose(1, 0, 2, 3)
    ).astype(BF)
    aug = np.concatenate([enc_core, np.ones((BL, S, 1), np.float32)], axis=2)
    enc_aug = np.ascontiguousarray(
        aug.reshape(BL, 8, 128, 257).transpose(2, 0, 1, 3)).astype(BF)
    return enc_t, enc_aug


# --------------------------------------------------------------------------
# device program
# --------------------------------------------------------------------------
def _build(flags, use_aff):
    nc = bacc.Bacc("TRN2", target_bir_lowering=False, debug=False)
    dt_of = {"bq_saT": F32, "b_qkT": F32, "posc": F32, "outw_t": F32, "outb": F32,
             "coordw": F32, "id2": F32, "lnaff": F32, "bf1T": F32}
    shapes = {
        "enc_t": [128, ET, BL, S], "enc_aug": [128, BL, 8, 257],
        "wq_sa": [128, L, ET, H * KD], "wk_sa": [128, L, ET, H * KD],
        "wv_sa": [128, L, ET, H * KD], "w_qk": [128, L, ET, H * E],
        "w1": [128, L, ET, MLPD],
        "wo_sa": [128, L, H * KD // 128, E], "w_vo": [128, L, H * E // 128, E],
        "w2": [128, L, MLPD // 128, E],
        "bq_saT": [128, L, 8], "b_qkT": [128, L, 8],
        "posc": [BL, T_MAX, E], "outw_t": [128, ET, 2], "outb": [2, 1],
        "coordw": [2, E], "id2": [2, 2], "id4": [4, 4],
    }
    if use_aff:
        shapes["lnaff"] = [BL, L, 5, E]
        shapes["bf1T"] = [128, L, 8]
    dr = {name: nc.dram_tensor(name, shp, dt_of.get(name, BF16), kind="ExternalInput")
          for name, shp in shapes.items()}
    out_dram = nc.dram_tensor("coords", [2, T_MAX, BL], F32, kind="ExternalOutput")

    with tile.TileContext(nc) as tc, ExitStack() as ctx:
        _emit(ctx, tc, nc, dr, out_dram, flags, use_aff)
    nc.compile()
    return nc


def _emit(ctx, tc, nc, dr, out_dram, flags, use_aff):
    persist = ctx.enter_context(tc.tile_pool(name="persist", bufs=1))
    sb = ctx.enter_context(tc.tile_pool(name="sb", bufs=3))
    # PSUM budget: 8 banks total.
    pbig = ctx.enter_context(tc.tile_pool(name="pbig", bufs=3, space="PSUM"))
    psmall = ctx.enter_context(tc.tile_pool(name="psmall", bufs=2, space="PSUM"))
    pce = ctx.enter_context(tc.tile_pool(name="pce", bufs=2, space="PSUM"))
    pct = ctx.enter_context(tc.tile_pool(name="pct", bufs=1, space="PSUM"))

    w = {}
    for name in dr:
        t = persist.tile(dr[name].shape, dr[name].dtype, tag=f"w_{name}")
        nc.sync.dma_start(out=t[:], in_=dr[name][:])
        w[name] = t

    eps_t = persist.tile([BL, 1], F32, tag="eps")
    nc.vector.memset(eps_t[:], EPS)
    ones128 = persist.tile([1, 128], F32, tag="ones128")
    nc.vector.memset(ones128[:], 1.0)

    x_sb = persist.tile([BL, E], F32, tag="x_sb")
    coordsT = persist.tile([2, T_MAX, BL], F32, tag="coordsT")
    kcT = persist.tile([128, L, 2, H, T_MAX, BL], BF16, tag="kcT")
    vcT = persist.tile([128, L, 2, H, T_MAX, BL], BF16, tag="vcT")

    nc.vector.tensor_copy(out=x_sb[:], in_=w["posc"][:, 0, :])

    def transpose_x(want_f32=False):
        xt_ps = pbig.tile([128, ET, BL], F32, tag="pbig")
        for et in range(ET):
            nc.tensor.matmul(
                xt_ps[:, et, :], lhsT=x_sb[:, et * 128:(et + 1) * 128],
                rhs=w["id2"][:], start=True, stop=True)
        xt = sb.tile([128, ET, BL], BF16, tag="xt")
        nc.scalar.copy(out=xt[:], in_=xt_ps[:])
        xtf = None
        if want_f32:
            xtf = sb.tile([128, ET, BL], F32, tag="xtf")
            nc.vector.tensor_copy(out=xtf[:], in_=xt_ps[:])
        return xt, xtf

    def residual_ln(add_ps, li, which):
        xr = sb.tile([BL, E], F32, tag="xr")
        nc.vector.tensor_add(out=xr[:], in0=x_sb[:], in1=add_ps[:])
        stats = sb.tile([BL, 6], F32, tag="stats")
        nc.vector.bn_stats(out=stats[:], in_=xr[:])
        mv = sb.tile([BL, 2], F32, tag="mv")
        nc.vector.bn_aggr(out=mv[:], in_=stats[:])
        rstd = sb.tile([BL, 1], F32, tag="rstd")
        nc.scalar.activation(out=rstd[:], in_=mv[:, 1:2], func=AF.Sqrt, bias=eps_t[:])
        nc.vector.reciprocal(out=rstd[:], in_=rstd[:])
        nc.vector.tensor_scalar(out=x_sb[:], in0=xr[:], scalar1=mv[:, 0:1],
                                scalar2=rstd[:], op0=ALU.subtract, op1=ALU.mult)
        if use_aff:
            g_on = flags["g1"][li] if which == 0 else flags["g2"][li]
            b_on = flags["b1"][li] if which == 0 else flags["b2"][li]
            gi, bi = (0, 1) if which == 0 else (2, 3)
            if g_on:
                nc.vector.tensor_mul(out=x_sb[:], in0=x_sb[:],
                                     in1=w["lnaff"][:, li, gi, :])
            if b_on:
                nc.vector.tensor_add(out=x_sb[:], in0=x_sb[:],
                                     in1=w["lnaff"][:, li, bi, :])

    for t in range(T_MAX):
        T = t + 1
        for li in range(L):
            # ============================ self attention =====================
            xt, _ = transpose_x()
            qkv_ps = pbig.tile([128, 3, 8, BL], F32, tag="pbig")
            for i, wn in enumerate(("wq_sa", "wk_sa", "wv_sa")):
                for m in range(8):
                    sl = slice(m * 128, (m + 1) * 128)
                    for et in range(ET):
                        nc.tensor.matmul(
                            qkv_ps[:, i, m, :], lhsT=w[wn][:, li, et, sl],
                            rhs=xt[:, et, :], start=et == 0, stop=et == ET - 1)
            qT = sb.tile([128, 8, BL], BF16, tag="qT")
            nc.vector.tensor_add(
                out=qT[:], in0=qkv_ps[:, 0, :, :],
                in1=w["bq_saT"][:, li, :].unsqueeze(2).to_broadcast((128, 8, BL)))
            for m in range(8):
                h, kdt = m // 2, m % 2
                if m % 2 == 0:
                    nc.vector.tensor_copy(out=kcT[:, li, kdt, h, t, :],
                                          in_=qkv_ps[:, 1, m, :])
                    nc.vector.tensor_copy(out=vcT[:, li, kdt, h, t, :],
                                          in_=qkv_ps[:, 2, m, :])
                else:
                    nc.scalar.copy(out=kcT[:, li, kdt, h, t, :],
                                   in_=qkv_ps[:, 1, m, :])
                    nc.scalar.copy(out=vcT[:, li, kdt, h, t, :],
                                   in_=qkv_ps[:, 2, m, :])
            l_ps = psmall.tile([1, BL * H, T_MAX], F32, tag="psmall")
            for b in range(BL):
                for h in range(H):
                    bh = b * H + h
                    for kdt in range(2):
                        nc.tensor.matmul(
                            l_ps[:, bh, 0:T], lhsT=qT[:, h * 2 + kdt, b:b + 1],
                            rhs=kcT[:, li, kdt, h, 0:T, b],
                            start=kdt == 0, stop=kdt == 1)
            expR = sb.tile([1, BL * H, T_MAX + 1], F32, tag="expR")
            if T < T_MAX:
                nc.vector.memset(expR[:], 0.0)
            nc.scalar.activation(out=expR[:, :, 0:T], in_=l_ps[:, :, 0:T], func=AF.Exp)
            nc.vector.reduce_sum(out=expR[:, :, T_MAX:T_MAX + 1],
                                 in_=expR[:, :, 0:T], axis=mybir.AxisListType.X)
            nc.vector.reciprocal(out=expR[:, :, T_MAX:T_MAX + 1],
                                 in_=expR[:, :, T_MAX:T_MAX + 1])
            abc_ps = pbig.tile([128, BL * H, T_MAX + 1], F32, tag="pbig")
            nc.tensor.matmul(abc_ps[:], lhsT=ones128[:], rhs=expR[:],
                             start=True, stop=True)
            ctxT = sb.tile([128, 2, H, BL], BF16, tag="ctxT")
            for b in range(BL):
                for h in range(H):
                    bh = b * H + h
                    avt = sb.tile([128, 2, T_MAX], F32, tag="avt")
                    nc.vector.tensor_mul(
                        out=avt[:, :, 0:T], in0=vcT[:, li, :, h, 0:T, b],
                        in1=abc_ps[:, bh, 0:T].unsqueeze(1).to_broadcast((128, 2, T)))
                    avr = sb.tile([128, 2], F32, tag="avr")
                    nc.vector.reduce_sum(out=avr[:].unsqueeze(2), in_=avt[:, :, 0:T],
                                         axis=mybir.AxisListType.X)
                    nc.vector.tensor_scalar_mul(
                        out=ctxT[:, :, h, b], in0=avr[:],
                        scalar1=abc_ps[:, bh, T_MAX:T_MAX + 1])
            ao_ps = psmall.tile([BL, E], F32, tag="psmall")
            for j in range(H * KD // 128):
                nc.tensor.matmul(ao_ps[:], lhsT=ctxT[:, j % 2, j // 2, :],
                                 rhs=w["wo_sa"][:, li, j, :], start=j == 0, stop=j == 7)
            residual_ln(ao_ps, li, 0)

            # ============================ cross attention ====================
            xt, _ = transpose_x()
            qt_ps = pbig.tile([128, 8, BL], F32, tag="pbig")
            for m in range(8):
                sl = slice(m * 128, (m + 1) * 128)
                for et in range(ET):
                    nc.tensor.matmul(qt_ps[:, m, :], lhsT=w["w_qk"][:, li, et, sl],
                                     rhs=xt[:, et, :], start=et == 0, stop=et == ET - 1)
            qt = sb.tile([128, ET, BL, H], BF16, tag="qt")
            nc.vector.tensor_add(
                out=qt[:].rearrange("p e b h -> p h e b"),
                in0=qt_ps[:].rearrange("p (h e) b -> p h e b", h=H),
                in1=w["b_qkT"][:, li, :].rearrange("p (h e) -> p h e", h=H)
                    .unsqueeze(3).to_broadcast((128, H, ET, BL)))
            lt_ps = pbig.tile([128, BL, 8, H], F32, tag="pbig")
            for b in range(BL):
                for sc in range(8):
                    for et in range(ET):
                        nc.tensor.matmul(
                            lt_ps[:, b, sc, :],
                            lhsT=w["enc_t"][:, et, b, sc * 128:(sc + 1) * 128],
                            rhs=qt[:, et, b, :], start=et == 0, stop=et == ET - 1)
            expt = sb.tile([128, BL, 8, H], BF16, tag="expt")
            nc.scalar.activation(out=expt[:], in_=lt_ps[:], func=AF.Exp)
            ce_ps = []
            for b in range(BL):
                cp = pce.tile([H, 257], F32, tag="pce")
                ce_ps.append(cp)
                for sc in range(8):
                    nc.tensor.matmul(cp[:], lhsT=expt[:, b, sc, :],
                                     rhs=w["enc_aug"][:, b, sc, :],
                                     start=sc == 0, stop=sc == 7)
            ct_ps = pct.tile([128, BL, ET, H], BF16, tag="pct")
            for b in range(BL):
                rc = sb.tile([H, 1], F32, tag="rc")
                nc.vector.reciprocal(out=rc[:], in_=ce_ps[b][:, 256:257])
                cn = sb.tile([H, E], BF16, tag="cn")
                nc.vector.tensor_scalar_mul(out=cn[:], in0=ce_ps[b][:, 0:256],
                                            scalar1=rc[:])
                for et in range(ET):
                    nc.tensor.transpose(out=ct_ps[:, b, et, :],
                                        in_=cn[:, et * 128:(et + 1) * 128],
                                        identity=w["id4"][:])
            ctf = sb.tile([128, ET, H, BL], BF16, tag="ctf")
            nc.vector.tensor_copy(out=ctf[:].rearrange("p e h b -> p b e h"),
                                  in_=ct_ps[:])
            oc_ps = psmall.tile([BL, E], F32, tag="psmall")
            for j in range(H * E // 128):
                nc.tensor.matmul(oc_ps[:], lhsT=ctf[:, j % 2, j // 2, :],
                                 rhs=w["w_vo"][:, li, j, :], start=j == 0, stop=j == 7)
            residual_ln(oc_ps, li, 1)

            # ================================ mlp ============================
            xt, _ = transpose_x()
            h1_ps = pbig.tile([128, 8, BL], F32, tag="pbig")
            for m in range(8):
                sl = slice(m * 128, (m + 1) * 128)
                for et in range(ET):
                    nc.tensor.matmul(h1_ps[:, m, :], lhsT=w["w1"][:, li, et, sl],
                                     rhs=xt[:, et, :], start=et == 0, stop=et == ET - 1)
            if use_aff and flags["bf1"][li]:
                nc.vector.tensor_add(
                    out=h1_ps[:], in0=h1_ps[:],
                    in1=w["bf1T"][:, li, :].unsqueeze(2).to_broadcast((128, 8, BL)))
            h1 = sb.tile([128, 8, BL], BF16, tag="h1")
            nc.scalar.activation(out=h1[:], in_=h1_ps[:], func=AF.Relu)
            mo_ps = psmall.tile([BL, E], F32, tag="psmall")
            for m in range(MLPD // 128):
                nc.tensor.matmul(mo_ps[:], lhsT=h1[:, m, :], rhs=w["w2"][:, li, m, :],
                                 start=m == 0, stop=m == 7)
            xr = sb.tile([BL, E], F32, tag="xr")
            nc.vector.tensor_add(out=xr[:], in0=x_sb[:], in1=mo_ps[:])
            if use_aff and flags["bf2"][li]:
                nc.vector.tensor_add(out=xr[:], in0=xr[:], in1=w["lnaff"][:, li, 4, :])
            nc.vector.tensor_copy(out=x_sb[:], in_=xr[:])

        # ==================== output proj + next token embedding =============
        _, xtf = transpose_x(want_f32=True)
        nc_ps = psmall.tile([2, BL], F32, tag="psmall")
        for et in range(ET):
            nc.tensor.matmul(nc_ps[:], lhsT=w["outw_t"][:, et, :], rhs=xtf[:, et, :],
                             start=et == 0, stop=et == ET - 1)
        nc.vector.tensor_scalar_add(out=coordsT[:, t, :], in0=nc_ps[:],
                                    scalar1=w["outb"][:])
        if t + 1 < T_MAX:
            xn_ps = psmall.tile([BL, E], F32, tag="psmall")
            nc.tensor.matmul(xn_ps[:], lhsT=coordsT[:, t, :], rhs=w["coordw"][:],
                             start=True, stop=True)
            nc.vector.tensor_add(out=x_sb[:], in0=xn_ps[:], in1=w["posc"][:, t + 1, :])

    nc.sync.dma_start(out=out_dram[:], in_=coordsT[:])


# --------------------------------------------------------------------------
# entry point
# --------------------------------------------------------------------------
_CACHE = {}


def kernel(encoder_output, params):
    enc = np.asarray(encoder_output, dtype=np.float32)
    weights, flags = _prep_weights(params)
    use_aff = any(any(v) for v in flags.values())
    key = tuple(tuple(bool(x) for x in v) for v in flags.values())
    if key not in _CACHE:
        _CACHE[key] = _build(flags, use_aff)
    nc = _CACHE[key]

    in_maps = []
    for c in range(NCORES):
        enc_t, enc_aug = _prep_enc(np.ascontiguousarray(enc[c * BL:(c + 1) * BL]))
        m = dict(weights)
        m["enc_t"] = enc_t
        m["enc_aug"] = enc_aug
        in_maps.append(m)

    res = run_bass_kernel_spmd(nc, in_maps, core_ids=list(range(NCORES)))
    # per-core result is [2(xy), T_MAX, BL] -> [BL, T_MAX, 2]
    return np.concatenate(
        [np.ascontiguousarray(r["coords"].transpose(2, 1, 0)) for r in res.results],
        axis=0).astype(np.float32)


# revision 17
# speedup vs baseline: 1.0596x; 1.0596x over previous
"""Trainium2 Bass kernel for nn_AutoregressiveDecoder.

Strategy
--------
* Math: the reference's full re-decode per step is exactly equivalent to
  incremental (KV-cached) decoding; additionally the encoder K/V projections
  fold into the query / output side of cross-attention:
      logits = x @ [(Wq/sqrt(KD)) @ Wk^T] @ Enc^T       (bk is softmax-shift invariant)
      ctx @ Wo = (A @ Enc) @ [Wv @ Wo]                  (bv folds via sum(A)=1, then LN-shift)
  so encoder K/V are never materialized; only Enc itself stays SBUF-resident
  (~3 MB/core in bf16).  A ones-column appended to Enc makes the matmul that
  computes A@Enc also produce the softmax denominators.
* Sharding: pure data parallel, batch 16 -> 2 per core x 8 cores, weights
  replicated, no collectives.
* Layouts: natural residual stream x [2, E] (bn_stats LayerNorm); transposed
  activations for matmul contraction produced by a tiny "x-stationary vs 2x2
  identity" matmul; q/k/v/MLP-hidden computed directly transposed
  (feature-on-partition) with bf16 weight tiles stationary (fast weight load).
"""
import os
import sys

import numpy as np

for _p in ("/opt/trn_rl_repo", "/root/.axon_site/_ro/trn_rl_repo"):
    if os.path.isdir(_p) and _p not in sys.path:
        sys.path.insert(0, _p)

from contextlib import ExitStack  # noqa: E402

import ml_dtypes  # noqa: E402

import concourse.bass as bass  # noqa: E402
import concourse.tile as tile  # noqa: E402
from concourse import bacc, mybir  # noqa: E402
from concourse.bass_utils import run_bass_kernel_spmd  # noqa: E402

B, S, E, H, KD, MLPD, T_MAX, L = 16, 1024, 256, 4, 256, 1024, 10, 2
NCORES = 8
BL = B // NCORES  # 2 batches per core
EPS = 1e-3
ET = E // 128     # contraction tiles over E
F32 = mybir.dt.float32
BF16 = mybir.dt.bfloat16
AF = mybir.ActivationFunctionType
ALU = mybir.AluOpType
BF = ml_dtypes.bfloat16


# --------------------------------------------------------------------------
# host-side weight folding
# --------------------------------------------------------------------------
def _f32(a):
    return np.ascontiguousarray(np.asarray(a, dtype=np.float32))


def _pack_rows(w, nkt):
    """[K, N] -> [128, nkt, N] so tile kt holds rows kt*128..kt*128+127."""
    return np.ascontiguousarray(w.reshape(nkt, 128, -1).transpose(1, 0, 2))


def _prep_weights(params):
    s = np.float32(1.0 / np.sqrt(KD))
    pos = _f32(params["pos"])
    coord_W = _f32(params["coord_W"])
    coord_b = _f32(params["coord_b"])
    out_W = _f32(params["out_W"])
    out_b = _f32(params["out_b"])

    acc = {k: [] for k in ("wq_sa", "wk_sa", "wv_sa", "w_qk", "w1",
                           "wo_sa", "w_vo", "w2", "bq_saT", "b_qkT")}
    gb = {k: [] for k in ("g1", "b1", "g2", "b2", "bf1", "bf2")}
    for lp in params["layers"]:
        sa = {k: _f32(v) for k, v in lp["sa"].items()}
        ca = {k: _f32(v) for k, v in lp["ca"].items()}
        acc["wq_sa"].append(_pack_rows((sa["Wq"] * s).reshape(E, H * KD), ET))
        acc["wk_sa"].append(_pack_rows(sa["Wk"].reshape(E, H * KD), ET))
        acc["wv_sa"].append(_pack_rows(sa["Wv"].reshape(E, H * KD), ET))
        acc["wo_sa"].append(_pack_rows(sa["Wo"].reshape(H * KD, E), H * KD // 128))
        acc["w_qk"].append(_pack_rows(
            np.einsum("ehd,fhd->ehf", ca["Wq"] * s, ca["Wk"]).reshape(E, H * E), ET))
        acc["w_vo"].append(_pack_rows(
            np.einsum("fhd,hde->hfe", ca["Wv"], ca["Wo"]).reshape(H * E, E),
            H * E // 128))
        acc["w1"].append(_pack_rows(_f32(lp["W1"]), ET))
        acc["w2"].append(_pack_rows(_f32(lp["W2"]), MLPD // 128))
        bq = (sa["bq"] * s).reshape(H * KD)
        acc["bq_saT"].append(np.ascontiguousarray(bq.reshape(H * KD // 128, 128).T))
        bqk = np.einsum("hd,fhd->hf", ca["bq"] * s, ca["Wk"]).reshape(H * E)
        acc["b_qkT"].append(np.ascontiguousarray(bqk.reshape(H * E // 128, 128).T))
        for k in gb:
            gb[k].append(_f32(lp[k]))

    out = {k: np.stack(v, 1).astype(BF) for k, v in acc.items()
           if k not in ("bq_saT", "b_qkT")}
    out["bq_saT"] = np.stack(acc["bq_saT"], 1).astype(np.float32)  # [128, L, 8]
    out["b_qkT"] = np.stack(acc["b_qkT"], 1).astype(np.float32)
    out["posc"] = np.ascontiguousarray(np.broadcast_to(
        (pos[:T_MAX] + coord_b[None, :])[None], (BL, T_MAX, E))).astype(np.float32)
    out["outw_t"] = np.ascontiguousarray(
        out_W.reshape(ET, 128, 2).transpose(1, 0, 2)).astype(np.float32)
    out["outb"] = out_b.reshape(2, 1).astype(np.float32)
    out["coordw"] = coord_W.astype(np.float32)
    out["id2"] = np.eye(2, dtype=np.float32)
    out["id2b"] = np.eye(2, dtype=BF)
    out["id4"] = np.eye(4, dtype=BF)

    flags = {
        "g1": [not np.allclose(g, 1.0) for g in gb["g1"]],
        "b1": [bool(np.any(b)) for b in gb["b1"]],
        "g2": [not np.allclose(g, 1.0) for g in gb["g2"]],
        "b2": [bool(np.any(b)) for b in gb["b2"]],
        "bf1": [bool(np.any(b)) for b in gb["bf1"]],
        "bf2": [bool(np.any(b)) for b in gb["bf2"]],
    }
    if any(any(v) for v in flags.values()):
        out["lnaff"] = np.ascontiguousarray(np.broadcast_to(np.stack(
            [np.stack([gb["g1"][l], gb["b1"][l], gb["g2"][l], gb["b2"][l],
                       gb["bf2"][l]]) for l in range(L)])[None],
            (BL, L, 5, E))).astype(np.float32)
        out["bf1T"] = np.stack(
            [np.ascontiguousarray(gb["bf1"][l].reshape(8, 128).T)
             for l in range(L)], 1).astype(np.float32)
    return out, flags


def _prep_enc(enc_core):
    """[BL, S, E] f32 -> enc_t [128, ET, BL, S] bf16 and enc_aug [128, BL, 8, 257] bf16."""
    enc_t = np.ascontiguousarray(
        enc_core.transpose(2, 0, 1).reshape(ET, 128, BL, S).transpose(1, 0, 2, 3)
    ).astype(BF)
    aug = np.concatenate([enc_core, np.ones((BL, S, 1), np.float32)], axis=2)
    enc_aug = np.ascontiguousarray(
        aug.reshape(BL, 8, 128, 257).transpose(2, 0, 1, 3)).astype(BF)
    return enc_t, enc_aug


# --------------------------------------------------------------------------
# device program
# --------------------------------------------------------------------------
def _build(flags, use_aff):
    nc = bacc.Bacc("TRN2", target_bir_lowering=False, debug=False)
    dt_of = {"bq_saT": F32, "b_qkT": F32, "posc": F32, "outw_t": F32, "outb": F32,
             "coordw": F32, "id2": F32, "lnaff": F32, "bf1T": F32}
    shapes = {
        "enc_t": [128, ET, BL, S], "enc_aug": [128, BL, 8, 257],
        "wq_sa": [128, L, ET, H * KD], "wk_sa": [128, L, ET, H * KD],
        "wv_sa": [128, L, ET, H * KD], "w_qk": [128, L, ET, H * E],
        "w1": [128, L, ET, MLPD],
        "wo_sa": [128, L, H * KD // 128, E], "w_vo": [128, L, H * E // 128, E],
        "w2": [128, L, MLPD // 128, E],
        "bq_saT": [128, L, 8], "b_qkT": [128, L, 8],
        "posc": [BL, T_MAX, E], "outw_t": [128, ET, 2], "outb": [2, 1],
        "coordw": [2, E], "id2": [2, 2], "id4": [4, 4],
    }
    dt_of["id2"] = F32
    shapes["id2b"] = [2, 2]
    if use_aff:
        shapes["lnaff"] = [BL, L, 5, E]
        shapes["bf1T"] = [128, L, 8]
    dr = {name: nc.dram_tensor(name, shp, dt_of.get(name, BF16), kind="ExternalInput")
          for name, shp in shapes.items()}
    out_dram = nc.dram_tensor("coords", [2, T_MAX, BL], F32, kind="ExternalOutput")

    with tile.TileContext(nc) as tc, ExitStack() as ctx:
        _emit(ctx, tc, nc, dr, out_dram, flags, use_aff)
    nc.compile()
    return nc


def _emit(ctx, tc, nc, dr, out_dram, flags, use_aff):
    persist = ctx.enter_context(tc.tile_pool(name="persist", bufs=1))
    sb = ctx.enter_context(tc.tile_pool(name="sb", bufs=3))
    # PSUM budget: 8 banks total.
    pbig = ctx.enter_context(tc.tile_pool(name="pbig", bufs=3, space="PSUM"))
    psmall = ctx.enter_context(tc.tile_pool(name="psmall", bufs=2, space="PSUM"))
    pce = ctx.enter_context(tc.tile_pool(name="pce", bufs=2, space="PSUM"))
    pct = ctx.enter_context(tc.tile_pool(name="pct", bufs=1, space="PSUM"))

    w = {}
    for name in dr:
        t = persist.tile(dr[name].shape, dr[name].dtype, tag=f"w_{name}")
        nc.sync.dma_start(out=t[:], in_=dr[name][:])
        w[name] = t

    eps_t = persist.tile([BL, 1], F32, tag="eps")
    nc.vector.memset(eps_t[:], EPS)
    ones128 = persist.tile([1, 128], F32, tag="ones128")
    nc.vector.memset(ones128[:], 1.0)

    x_sb = persist.tile([BL, E], F32, tag="x_sb")
    coordsT = persist.tile([2, T_MAX, BL], F32, tag="coordsT")
    kcT = persist.tile([128, L, 2, H, T_MAX, BL], BF16, tag="kcT")
    vcT = persist.tile([128, L, 2, H, T_MAX, BL], BF16, tag="vcT")

    nc.vector.tensor_copy(out=x_sb[:], in_=w["posc"][:, 0, :])

    def transpose_x(want_f32=False):
        xt_ps = pbig.tile([128, ET, BL], F32, tag="pbig")
        if want_f32:
            # exact fp32 path (used by the output projection only)
            for et in range(ET):
                nc.tensor.matmul(
                    xt_ps[:, et, :], lhsT=x_sb[:, et * 128:(et + 1) * 128],
                    rhs=w["id2"][:], start=True, stop=True)
            xtf = sb.tile([128, ET, BL], F32, tag="xtf")
            nc.vector.tensor_copy(out=xtf[:], in_=xt_ps[:])
            xt = sb.tile([128, ET, BL], BF16, tag="xt")
            nc.scalar.copy(out=xt[:], in_=xt_ps[:])
            return xt, xtf
        # bf16 path: cast first so the PE stream stays pure bf16
        xb = sb.tile([BL, E], BF16, tag="xb")
        nc.vector.tensor_copy(out=xb[:], in_=x_sb[:])
        for et in range(ET):
            nc.tensor.matmul(
                xt_ps[:, et, :], lhsT=xb[:, et * 128:(et + 1) * 128],
                rhs=w["id2b"][:], start=True, stop=True)
        xt = sb.tile([128, ET, BL], BF16, tag="xt")
        nc.scalar.copy(out=xt[:], in_=xt_ps[:])
        return xt, None

    def residual_ln(add_ps, li, which):
        xr = sb.tile([BL, E], F32, tag="xr")
        nc.vector.tensor_add(out=xr[:], in0=x_sb[:], in1=add_ps[:])
        stats = sb.tile([BL, 6], F32, tag="stats")
        nc.vector.bn_stats(out=stats[:], in_=xr[:])
        mv = sb.tile([BL, 2], F32, tag="mv")
        nc.vector.bn_aggr(out=mv[:], in_=stats[:])
        rstd = sb.tile([BL, 1], F32, tag="rstd")
        nc.scalar.activation(out=rstd[:], in_=mv[:, 1:2], func=AF.Sqrt, bias=eps_t[:])
        nc.vector.reciprocal(out=rstd[:], in_=rstd[:])
        nc.vector.tensor_scalar(out=x_sb[:], in0=xr[:], scalar1=mv[:, 0:1],
                                scalar2=rstd[:], op0=ALU.subtract, op1=ALU.mult)
        if use_aff:
            g_on = flags["g1"][li] if which == 0 else flags["g2"][li]
            b_on = flags["b1"][li] if which == 0 else flags["b2"][li]
            gi, bi = (0, 1) if which == 0 else (2, 3)
            if g_on:
                nc.vector.tensor_mul(out=x_sb[:], in0=x_sb[:],
                                     in1=w["lnaff"][:, li, gi, :])
            if b_on:
                nc.vector.tensor_add(out=x_sb[:], in0=x_sb[:],
                                     in1=w["lnaff"][:, li, bi, :])

    for t in range(T_MAX):
        T = t + 1
        for li in range(L):
            # ============================ self attention =====================
            xt, _ = transpose_x()
            qkv_ps = pbig.tile([128, 3, 8, BL], F32, tag="pbig")
            for i, wn in enumerate(("wq_sa", "wk_sa", "wv_sa")):
                for m in range(8):
                    sl = slice(m * 128, (m + 1) * 128)
                    for et in range(ET):
                        nc.tensor.matmul(
                            qkv_ps[:, i, m, :], lhsT=w[wn][:, li, et, sl],
                            rhs=xt[:, et, :], start=et == 0, stop=et == ET - 1)
            qT = sb.tile([128, 8, BL], BF16, tag="qT")
            nc.vector.tensor_add(
                out=qT[:], in0=qkv_ps[:, 0, :, :],
                in1=w["bq_saT"][:, li, :].unsqueeze(2).to_broadcast((128, 8, BL)))
            for m in range(8):
                h, kdt = m // 2, m % 2
                if m % 2 == 0:
                    nc.vector.tensor_copy(out=kcT[:, li, kdt, h, t, :],
                                          in_=qkv_ps[:, 1, m, :])
                    nc.vector.tensor_copy(out=vcT[:, li, kdt, h, t, :],
                                          in_=qkv_ps[:, 2, m, :])
                else:
                    nc.scalar.copy(out=kcT[:, li, kdt, h, t, :],
                                   in_=qkv_ps[:, 1, m, :])
                    nc.scalar.copy(out=vcT[:, li, kdt, h, t, :],
                                   in_=qkv_ps[:, 2, m, :])
            l_ps = psmall.tile([1, BL * H, T_MAX], F32, tag="psmall")
            for b in range(BL):
                for h in range(H):
                    bh = b * H + h
                    for kdt in range(2):
                        nc.tensor.matmul(
                            l_ps[:, bh, 0:T], lhsT=qT[:, h * 2 + kdt, b:b + 1],
                            rhs=kcT[:, li, kdt, h, 0:T, b],
                            start=kdt == 0, stop=kdt == 1)
            expR = sb.tile([1, BL * H, T_MAX + 1], F32, tag="expR")
            if T < T_MAX:
                nc.vector.memset(expR[:], 0.0)
            nc.scalar.activation(out=expR[:, :, 0:T], in_=l_ps[:, :, 0:T], func=AF.Exp)
            nc.vector.reduce_sum(out=expR[:, :, T_MAX:T_MAX + 1],
                                 in_=expR[:, :, 0:T], axis=mybir.AxisListType.X)
            nc.vector.reciprocal(out=expR[:, :, T_MAX:T_MAX + 1],
                                 in_=expR[:, :, T_MAX:T_MAX + 1])
            abc_ps = pbig.tile([128, BL * H, T_MAX + 1], F32, tag="pbig")
            nc.tensor.matmul(abc_ps[:], lhsT=ones128[:], rhs=expR[:],
                             start=True, stop=True)
            ctxT = sb.tile([128, 2, H, BL], BF16, tag="ctxT")
            for b in range(BL):
                for h in range(H):
                    bh = b * H + h
                    avt = sb.tile([128, 2, T_MAX], F32, tag="avt")
                    nc.vector.tensor_mul(
                        out=avt[:, :, 0:T], in0=vcT[:, li, :, h, 0:T, b],
                        in1=abc_ps[:, bh, 0:T].unsqueeze(1).to_broadcast((128, 2, T)))
                    avr = sb.tile([128, 2], F32, tag="avr")
                    nc.vector.reduce_sum(out=avr[:].unsqueeze(2), in_=avt[:, :, 0:T],
                                         axis=mybir.AxisListType.X)
                    nc.vector.tensor_scalar_mul(
                        out=ctxT[:, :, h, b], in0=avr[:],
                        scalar1=abc_ps[:, bh, T_MAX:T_MAX + 1])
            ao_ps = psmall.tile([BL, E], F32, tag="psmall")
            for j in range(H * KD // 128):
                nc.tensor.matmul(ao_ps[:], lhsT=ctxT[:, j % 2, j // 2, :],
                                 rhs=w["wo_sa"][:, li, j, :], start=j == 0, stop=j == 7)
            residual_ln(ao_ps, li, 0)

            # ============================ cross attention ====================
            xt, _ = transpose_x()
            qt_ps = pbig.tile([128, 8, BL], F32, tag="pbig")
            for m in range(8):
                sl = slice(m * 128, (m + 1) * 128)
                for et in range(ET):
                    nc.tensor.matmul(qt_ps[:, m, :], lhsT=w["w_qk"][:, li, et, sl],
                                     rhs=xt[:, et, :], start=et == 0, stop=et == ET - 1)
            qt = sb.tile([128, ET, BL, H], BF16, tag="qt")
            nc.vector.tensor_add(
                out=qt[:].rearrange("p e b h -> p h e b"),
                in0=qt_ps[:].rearrange("p (h e) b -> p h e b", h=H),
                in1=w["b_qkT"][:, li, :].rearrange("p (h e) -> p h e", h=H)
                    .unsqueeze(3).to_broadcast((128, H, ET, BL)))
            lt_ps = pbig.tile([128, BL, 8, H], F32, tag="pbig")
            for b in range(BL):
                for sc in range(8):
                    for et in range(ET):
                        nc.tensor.matmul(
                            lt_ps[:, b, sc, :],
                            lhsT=w["enc_t"][:, et, b, sc * 128:(sc + 1) * 128],
                            rhs=qt[:, et, b, :], start=et == 0, stop=et == ET - 1)
            expt = sb.tile([128, BL, 8, H], BF16, tag="expt")
            nc.scalar.activation(out=expt[:], in_=lt_ps[:], func=AF.Exp)
            ce_ps = []
            for b in range(BL):
                cp = pce.tile([H, 257], F32, tag="pce")
                ce_ps.append(cp)
                for sc in range(8):
                    nc.tensor.matmul(cp[:], lhsT=expt[:, b, sc, :],
                                     rhs=w["enc_aug"][:, b, sc, :],
                                     start=sc == 0, stop=sc == 7)
            ct_ps = pct.tile([128, BL, ET, H], BF16, tag="pct")
            for b in range(BL):
                rc = sb.tile([H, 1], F32, tag="rc")
                nc.vector.reciprocal(out=rc[:], in_=ce_ps[b][:, 256:257])
                cn = sb.tile([H, E], BF16, tag="cn")
                nc.vector.tensor_scalar_mul(out=cn[:], in0=ce_ps[b][:, 0:256],
                                            scalar1=rc[:])
                for et in range(ET):
                    nc.tensor.transpose(out=ct_ps[:, b, et, :],
                                        in_=cn[:, et * 128:(et + 1) * 128],
                                        identity=w["id4"][:])
            ctf = sb.tile([128, ET, H, BL], BF16, tag="ctf")
            nc.vector.tensor_copy(out=ctf[:].rearrange("p e h b -> p b e h"),
                                  in_=ct_ps[:])
            oc_ps = psmall.tile([BL, E], F32, tag="psmall")
            for j in range(H * E // 128):
                nc.tensor.matmul(oc_ps[:], lhsT=ctf[:, j % 2, j // 2, :],
                                 rhs=w["w_vo"][:, li, j, :], start=j == 0, stop=j == 7)
            residual_ln(oc_ps, li, 1)

            # ================================ mlp ============================
            xt, _ = transpose_x()
            h1_ps = pbig.tile([128, 8, BL], F32, tag="pbig")
            for m in range(8):
                sl = slice(m * 128, (m + 1) * 128)
                for et in range(ET):
                    nc.tensor.matmul(h1_ps[:, m, :], lhsT=w["w1"][:, li, et, sl],
                                     rhs=xt[:, et, :], start=et == 0, stop=et == ET - 1)
            if use_aff and flags["bf1"][li]:
                nc.vector.tensor_add(
                    out=h1_ps[:], in0=h1_ps[:],
                    in1=w["bf1T"][:, li, :].unsqueeze(2).to_broadcast((128, 8, BL)))
            h1 = sb.tile([128, 8, BL], BF16, tag="h1")
            nc.vector.tensor_scalar_max(out=h1[:], in0=h1_ps[:], scalar1=0.0)
            mo_ps = psmall.tile([BL, E], F32, tag="psmall")
            for m in range(MLPD // 128):
                nc.tensor.matmul(mo_ps[:], lhsT=h1[:, m, :], rhs=w["w2"][:, li, m, :],
                                 start=m == 0, stop=m == 7)
            xr = sb.tile([BL, E], F32, tag="xr")
            nc.vector.tensor_add(out=xr[:], in0=x_sb[:], in1=mo_ps[:])
            if use_aff and flags["bf2"][li]:
                nc.vector.tensor_add(out=xr[:], in0=xr[:], in1=w["lnaff"][:, li, 4, :])
            nc.vector.tensor_copy(out=x_sb[:], in_=xr[:])

        # ==================== output proj + next token embedding =============
        _, xtf = transpose_x(want_f32=True)
        nc_ps = psmall.tile([2, BL], F32, tag="psmall")
        for et in range(ET):
            nc.tensor.matmul(nc_ps[:], lhsT=w["outw_t"][:, et, :], rhs=xtf[:, et, :],
                             start=et == 0, stop=et == ET - 1)
        nc.vector.tensor_scalar_add(out=coordsT[:, t, :], in0=nc_ps[:],
                                    scalar1=w["outb"][:])
        if t + 1 < T_MAX:
            xn_ps = psmall.tile([BL, E], F32, tag="psmall")
            nc.tensor.matmul(xn_ps[:], lhsT=coordsT[:, t, :], rhs=w["coordw"][:],
                             start=True, stop=True)
            nc.vector.tensor_add(out=x_sb[:], in0=xn_ps[:], in1=w["posc"][:, t + 1, :])

    nc.sync.dma_start(out=out_dram[:], in_=coordsT[:])


# --------------------------------------------------------------------------
# entry point
# --------------------------------------------------------------------------
_CACHE = {}


def kernel(encoder_output, params):
    enc = np.asarray(encoder_output, dtype=np.float32)
    weights, flags = _prep_weights(params)
    use_aff = any(any(v) for v in flags.values())
    key = tuple(tuple(bool(x) for x in v) for v in flags.values())
    if key not in _CACHE:
        _CACHE[key] = _build(flags, use_aff)
    nc = _CACHE[key]

    in_maps = []
    for c in range(NCORES):
        enc_t, enc_aug = _prep_enc(np.ascontiguousarray(enc[c * BL:(c + 1) * BL]))
        m = dict(weights)
        m["enc_t"] = enc_t
        m["enc_aug"] = enc_aug
        in_maps.append(m)

    res = run_bass_kernel_spmd(nc, in_maps, core_ids=list(range(NCORES)))
    # per-core result is [2(xy), T_MAX, BL] -> [BL, T_MAX, 2]
    return np.concatenate(
        [np.ascontiguousarray(r["coords"].transpose(2, 1, 0)) for r in res.results],
        axis=0).astype(np.float32)
